# revision 17
# baseline (speedup 1.0000x reference)
"""Trainium2 Bass kernel for nn_EulerEquationModel (Euler equation step).

Contract: kernel(**inputs) takes FULL inputs (Field (4,12,720,1440) f32,
W (15,12), b_bias (15,), thermal_factor (1,)) and returns the full output
tuple matching reference():
  (Field_new (4,12,720,1440) f32, mean(constrain^2), mean(E^2),
   mean(PhysicsPart^2), mean(xydir^2))

Sharding: 8 cores = 4 batches x 2 y-halves (360 rows each + 2-row circular
y-halo supplied by the host). All params replicated.

On-chip plan per core (shard F: (12, 364, 1440) f32):
  - y-blocks of 124 rows (output rows 2..121 of each block), x-chunks of
    720+4 halo columns (circular x-halo in-tile).
  - PE (fp16 matmuls): Fdy via banded stencil lhsT; Fdz via +/-0.5*I over
    z-neighbor channel tiles; channel mix (W+I fold) in an interleaved
    (yl, c) partition layout, 8 y-rows per matmul, bias folded into the
    PSUM->SBUF activation copy.
  - DVE/GpSimd (fp16): x-derivative from shifted access patterns (xs16 is
    a 1-element-shifted physical copy so odd shifts stay 4B-aligned),
    advection products and sums, square-sum reductions via
    tensor_tensor_reduce accum_out.
  - Means are reduced to per-partition partial sums on-chip and finished
    on the host in float64.
"""

import numpy as np

import concourse.bacc as bacc
import concourse.mybir as mybir
import concourse.tile as tile
from concourse import bass_utils
import concourse.bass as bass

AOT = mybir.AluOpType
AFT = mybir.ActivationFunctionType
F32 = mybir.dt.float32
F16 = mybir.dt.float16

B, C, Y, X = 4, 12, 720, 1440
NCORES = 8
YS = Y // 2          # 360 rows per core
YSH = YS + 4         # 364 shard rows (2-row halo each side)
NBLK = 3             # y-blocks per core
BLK = 120            # output rows per block
TP = 124             # tile partitions (BLK + 4 halo rows)
NCH = 3              # x-chunks
CHK = X // NCH       # 720
CW = CHK + 4         # 724 tile cols (2-col halo each side)
NO = 15              # mix output channels (12 Enew + 3 o)
MIXK = 96            # mix contraction: 8 yl * 12 c
MIXM = 120           # mix out: 8 yl * 15 o
NSB = BLK // 8       # 15 sub-blocks per block

# square-sum partial column layout: kind -> base column
NPER = NBLK * NCH * C          # 72 cols per 12-channel kind
COL_XY, COL_PP, COL_E = 0, NPER, 2 * NPER
COL_CON = 3 * NPER             # 18 cols (3 z * 6 block-chunks)
NCOLS = 3 * NPER + NBLK * NCH * 3

_CACHE = {}


def _host_consts(W, b_bias, thermal_factor):
    w5a, w5b = 8.0 / 12.0, 1.0 / 12.0  # d1, d2 coefficients (Fdx = w5a*d1 - w5b*d2)
    # mix lhsT [96, 120]: (yl*12+c, yl*15+o) = W[o,c] + I(o==c)
    mix = np.zeros((MIXK, MIXM), np.float32)
    for yl in range(8):
        for o in range(NO):
            for c in range(C):
                v = W[o, c] + (1.0 if o == c else 0.0)
                mix[yl * 12 + c, yl * 15 + o] = v
    # Fdy banded stencil lhsT [TP, BLK]. rhs partition k holds block row
    # r(k): k<BLK -> 2+k (valid rows), k=120..123 -> halo rows 0,1,122,123.
    # out column m is block row 2+m.
    rowmap = list(range(2, 2 + BLK)) + [0, 1, BLK + 2, BLK + 3]
    sfy = np.zeros((TP, BLK), np.float32)
    taps = {-2: w5b, -1: -w5a, 1: w5a, 2: -w5b}
    for k in range(TP):
        for m in range(BLK):
            d = rowmap[k] - (2 + m)
            if d in taps:
                sfy[k, m] = taps[d]
    halfI = np.zeros((TP, BLK), np.float32)
    for m in range(BLK):
        halfI[m, m] = 0.5
    b_aug = np.zeros((MIXM, 1), np.float32)
    for yl in range(8):
        for o in range(NO):
            b_aug[yl * 15 + o, 0] = b_bias[o]
    p_list = np.array([10.0, 8.5, 5.0], np.float32)
    tfp = np.tile((thermal_factor[0] / p_list)[None, :], (TP, 1)).astype(np.float32)
    return {
        "lhsT_mix": mix.astype(np.float16),
        "lhsT_sfy": sfy.astype(np.float16),
        "lhsT_hI": halfI.astype(np.float16),
        "lhsT_nhI": (-halfI).astype(np.float16),
        "b_aug": b_aug,
        "tfp": tfp,
    }


def _zc(z, d):
    return (z + d) % 3


def build_nc():
    """Build and compile the per-core Bass program (SPMD: same on all cores)."""
    nc = bacc.Bacc("TRN2", target_bir_lowering=False, debug=False,
                   num_devices=NCORES)

    F = nc.dram_tensor("F", [C, YSH, X], F32, kind="ExternalInput").ap()
    lhsT_mix = nc.dram_tensor("lhsT_mix", [MIXK, MIXM], F16, kind="ExternalInput").ap()
    lhsT_sfy = nc.dram_tensor("lhsT_sfy", [TP, BLK], F16, kind="ExternalInput").ap()
    lhsT_hI = nc.dram_tensor("lhsT_hI", [TP, BLK], F16, kind="ExternalInput").ap()
    lhsT_nhI = nc.dram_tensor("lhsT_nhI", [TP, BLK], F16, kind="ExternalInput").ap()
    b_aug_d = nc.dram_tensor("b_aug", [MIXM, 1], F32, kind="ExternalInput").ap()
    tfp_d = nc.dram_tensor("tfp", [TP, 3], F32, kind="ExternalInput").ap()

    FN = nc.dram_tensor("FN", [C, YS, X], F32, kind="ExternalOutput").ap()
    PART = nc.dram_tensor("PART", [TP, NCOLS], F32, kind="ExternalOutput").ap()

    with tile.TileContext(nc) as tc:
        with (
            tc.tile_pool(name="consts", bufs=1) as cpool,
            tc.tile_pool(name="xin", bufs=4) as xpool,
            tc.tile_pool(name="x16p", bufs=1) as x16pool,
            tc.tile_pool(name="l2p", bufs=2) as l2pool,
            tc.tile_pool(name="ml2", bufs=1) as ml2pool,
            tc.tile_pool(name="big16", bufs=1) as bigpool,
            tc.tile_pool(name="tmp", bufs=2) as tpool,
            tc.tile_pool(name="fnp", bufs=3) as fnpool,
            tc.tile_pool(name="partp", bufs=1) as partpool,
            tc.tile_pool(name="dramp", bufs=2, space="DRAM") as dramp,
            tc.tile_pool(name="ps_fy", bufs=2, space="PSUM") as psfy,
            tc.tile_pool(name="ps_fz", bufs=2, space="PSUM") as psfz,
            tc.tile_pool(name="ps_mx", bufs=2, space="PSUM") as psmx,
        ):
            # constants
            c_mix = cpool.tile([MIXK, MIXM], F16, tag="c_mix")
            nc.sync.dma_start(c_mix[:], lhsT_mix[:])
            c_sfy = cpool.tile([TP, BLK], F16, tag="c_sfy")
            nc.sync.dma_start(c_sfy[:], lhsT_sfy[:])
            c_hI = cpool.tile([TP, BLK], F16, tag="c_hI")
            nc.sync.dma_start(c_hI[:], lhsT_hI[:])
            c_nhI = cpool.tile([TP, BLK], F16, tag="c_nhI")
            nc.sync.dma_start(c_nhI[:], lhsT_nhI[:])
            c_bias = cpool.tile([MIXM, 1], F32, tag="c_bias")
            nc.sync.dma_start(c_bias[:], b_aug_d[:])
            c_tfp = cpool.tile([TP, 3], F32, tag="c_tfp")
            nc.sync.dma_start(c_tfp[:], tfp_d[:])

            part = partpool.tile([TP, NCOLS], F32, tag="part")
            nc.vector.memset(part[:], 0.0)

            for j in range(NBLK):
                y0 = j * BLK  # shard row of tile partition 0
                for i in range(NCH):
                    x0 = i * CHK  # global x of tile col 2
                    blk = j * NCH + i

                    # ---- L2 mix input: (yl,c)-partitions, fp16 cast DMA ----
                    l2 = l2pool.tile([MIXK, NSB, CHK], F16, tag="l2")
                    # partition yl*12+c holds F[c, y0+2+sb*8+yl, x] per (sb, x);
                    # one DMA per yl -> contiguous 12-partition writes
                    for yl in range(8):
                        nc.gpsimd.dma_start(
                            l2[yl * C: (yl + 1) * C],
                            F[:, y0 + 2 + yl: y0 + 2 + BLK: 8, x0: x0 + CHK],
                        )

                    # ---- per-channel fp32 load + fp16 casts ----
                    x16 = x16pool.tile([TP, C, CW], F16, tag="x16")
                    xs16 = x16pool.tile([TP, C, CW], F16, tag="xs16")
                    for c in range(C):
                        x32 = xpool.tile([TP, CW], F32, tag="x32")
                        # partitions 0..119 = block rows 2..121 (valid),
                        # partitions 120..123 = halo rows 0,1,122,123
                        for p0, r0, nr in ((0, y0 + 2, BLK), (BLK, y0, 2),
                                           (BLK + 2, y0 + BLK + 2, 2)):
                            if i == 0:
                                nc.sync.dma_start(
                                    x32[p0:p0 + nr, 2:CW],
                                    F[c, r0: r0 + nr, 0: CHK + 2])
                                nc.sync.dma_start(
                                    x32[p0:p0 + nr, 0:2],
                                    F[c, r0: r0 + nr, X - 2: X])
                            elif i == NCH - 1:
                                nc.sync.dma_start(
                                    x32[p0:p0 + nr, 0: CW - 2],
                                    F[c, r0: r0 + nr, x0 - 2: x0 + CHK])
                                nc.sync.dma_start(
                                    x32[p0:p0 + nr, CW - 2: CW],
                                    F[c, r0: r0 + nr, 0:2])
                            else:
                                nc.sync.dma_start(
                                    x32[p0:p0 + nr, :],
                                    F[c, r0: r0 + nr, x0 - 2: x0 + CHK + 2])
                        nc.vector.tensor_copy(x16[:, c, :], x32[:])
                        nc.vector.tensor_copy(xs16[:, c, 0: CW - 2], x32[:, 1: CW - 1])

                    # ---- PE: channel mix in 8-row sub-blocks ----
                    enew16 = bigpool.tile([TP, C, CHK], F16, tag="enew16",
                                          padded_shape=[TP, C + 1, CHK])
                    o16 = bigpool.tile([TP, 3, CHK], F16, tag="o16",
                                       padded_shape=[TP, 4, CHK])
                    strip = ml2pool.tile([MIXM, NSB, CHK], F16, tag="strip")
                    for sb in range(NSB):
                        mps = psmx.tile([MIXM, 2, CHK // 2], F32, tag="mps")
                        rhs = l2[:, sb, :]
                        for h in range(2):
                            nc.tensor.matmul(
                                mps[:, h, :], c_mix[:],
                                rhs[:, h * (CHK // 2): (h + 1) * (CHK // 2)],
                                start=True, stop=True)
                        nc.scalar.activation(
                            strip[:, sb, :], mps[:].rearrange("p b x -> p (b x)"),
                            AFT.Identity, bias=c_bias[:], scale=1.0)
                    # remap (yl*15+o) partitions -> y-partition channel tiles
                    # via a DRAM scratch round-trip (fp16). SBUF->SBUF
                    # partition remaps need exotic access patterns that the
                    # Tile dependency tracker cannot follow; DRAM-side APs
                    # are unconstrained, so both hops use plain APs.
                    e2 = dramp.tile([NO, BLK, CHK], F16, tag="e2")
                    e2r = e2[:].rearrange("o (sb yl) x -> yl o sb x", yl=8)
                    for yl in range(8):
                        nc.sync.dma_start(
                            e2r[yl], strip[yl * NO: (yl + 1) * NO, :, :])
                    nc.sync.dma_start(
                        enew16[0:BLK], e2[0:C].rearrange("c y x -> y c x"))
                    nc.sync.dma_start(
                        o16[0:BLK], e2[C:NO].rearrange("z y x -> y z x"))

                    # ---- PE: Fdy + Fdz per channel; stencil prep; chain ----
                    fdy16 = bigpool.tile([TP, C, CHK], F16, tag="fdy16")
                    fdz16 = bigpool.tile([TP, C, CHK], F16, tag="fdz16")
                    gneg = bigpool.tile([TP, C, CHK], F16, tag="gneg")
                    d1t = tpool.tile([TP, CHK], F16, tag="d1")
                    d2t = tpool.tile([TP, CHK], F16, tag="d2")
                    for c in range(C):
                        # Fdy
                        fyp = psfy.tile([BLK, 2, CHK // 2], F32, tag="fyp")
                        for h in range(2):
                            nc.tensor.matmul(
                                fyp[:, h, :], c_sfy[:],
                                x16[:, c, 2 + h * (CHK // 2): 2 + (h + 1) * (CHK // 2)],
                                start=True, stop=True)
                        nc.scalar.activation(
                            fdy16[0:BLK, c, :], fyp[:].rearrange("p b x -> p (b x)"),
                            AFT.Copy)
                        # Fdz = 0.5*(x[zp] - x[zm])
                        s, z = c // 3, c % 3
                        zp = 3 * s + _zc(z, 1)
                        zm = 3 * s + _zc(z, -1)
                        fzp = psfz.tile([BLK, 2, CHK // 2], F32, tag="fzp")
                        for h in range(2):
                            w = slice(2 + h * (CHK // 2), 2 + (h + 1) * (CHK // 2))
                            nc.tensor.matmul(fzp[:, h, :], c_hI[:], x16[:, zp, w],
                                             start=True, stop=False)
                            nc.tensor.matmul(fzp[:, h, :], c_nhI[:], x16[:, zm, w],
                                             start=False, stop=True)
                        nc.scalar.activation(
                            fdz16[0:BLK, c, :], fzp[:].rearrange("p b x -> p (b x)"),
                            AFT.Copy)
                        # x-derivative pieces: d1 = x[+1]-x[-1], d2 = x[+2]-x[-2]
                        nc.vector.tensor_sub(
                            d1t[:], xs16[:, c, 2:CW - 2], xs16[:, c, 0:CW - 4])
                        nc.gpsimd.tensor_sub(
                            d2t[:], x16[:, c, 4:CW], x16[:, c, 0:CW - 4])
                        # gneg = d2/8 - d1  (= -(3/2)*Fdx scale; Fdx = -(2/3)*gneg)
                        nc.vector.scalar_tensor_tensor(
                            gneg[:, c, :], d2t[:], 0.125, d1t[:],
                            AOT.mult, AOT.subtract)

                    # ---- prep: uneg = -(2/3)u, term3 = (tf/p)*T*o ----
                    uneg = tpool.tile([TP, 3, CHK], F16, tag="uneg")
                    nc.vector.tensor_scalar_mul(
                        uneg[:], x16[:, 0:3, 2:CW - 2], -2.0 / 3.0)
                    term3 = tpool.tile([TP, 3, CHK], F16, tag="term3")
                    for z in range(3):
                        tprime = tpool.tile([TP, CHK], F16, tag="tprime")
                        nc.vector.tensor_scalar(
                            tprime[0:BLK], x16[0:BLK, 6 + z, 2:CW - 2],
                            c_tfp[0:BLK, z:z+1], None, AOT.mult)
                        nc.vector.tensor_mul(
                            term3[0:BLK, z, :], tprime[0:BLK], o16[0:BLK, z, :])

                    # ---- nonlinear chain per channel ----
                    V = slice(0, BLK)  # valid partition rows
                    for c in range(C):
                        s, z = c // 3, c % 3
                        A = tpool.tile([TP, CHK], F16, tag="A")
                        nc.vector.tensor_mul(A[V], gneg[V, c, :], uneg[V, z, :])
                        Bt = tpool.tile([TP, CHK], F16, tag="Bt")
                        nc.vector.tensor_mul(
                            Bt[V], fdy16[V, c, :], x16[V, 3 + z, 2:CW - 2])
                        S1 = tpool.tile([TP, CHK], F16, tag="S1")
                        nc.vector.tensor_add(S1[V], A[V], Bt[V])
                        Ct = tpool.tile([TP, CHK], F16, tag="Ct")
                        nc.vector.tensor_mul(Ct[V], fdz16[V, c, :], o16[V, z, :])
                        q2 = tpool.tile([TP, CHK], F16, tag="q2")
                        nc.vector.tensor_add(q2[V], S1[V], Ct[V])
                        trash = tpool.tile([TP, CHK], F16, tag="trash")
                        # r1: sum(xydir^2) = sum(S1^2)
                        nc.scalar.activation(
                            trash[V], S1[V], AFT.Square,
                            accum_out=part[V, COL_XY + blk * C + c: COL_XY + blk * C + c + 1])
                        # q3 = q2 - phys_c
                        if s == 0:
                            q3 = tpool.tile([TP, CHK], F16, tag="q3")
                            nc.vector.scalar_tensor_tensor(
                                q3[V], gneg[V, 9 + z, :], -2.0 / 3.0, q2[V],
                                AOT.mult, AOT.add)
                        elif s == 1:
                            q3 = tpool.tile([TP, CHK], F16, tag="q3")
                            nc.vector.tensor_add(q3[V], q2[V], fdy16[V, 9 + z, :])
                        elif s == 2:
                            q3 = tpool.tile([TP, CHK], F16, tag="q3")
                            nc.vector.tensor_sub(q3[V], q2[V], term3[V, z, :])
                        else:
                            q3 = q2
                        # r2: sum(PhysicsPart^2) = sum(q3^2)
                        nc.vector.scalar_tensor_tensor(
                            trash[V], q3[V], 1.0, q3[V], AOT.mult, AOT.mult,
                            accum_out=part[V, COL_PP + blk * C + c: COL_PP + blk * C + c + 1])
                        # FN = Enew - q3 (fp32 out)
                        fn = fnpool.tile([TP, CHK], F32, tag="fn")
                        nc.vector.tensor_sub(fn[V], enew16[V, c, :], q3[V])
                        nc.sync.dma_start(
                            FN[c, y0: y0 + BLK, x0: x0 + CHK], fn[V])
                        # r3: sum(E^2), E = Enew - F
                        e16 = tpool.tile([TP, CHK], F16, tag="e16")
                        nc.vector.tensor_sub(
                            e16[V], enew16[V, c, :], x16[V, c, 2:CW - 2])
                        nc.scalar.activation(
                            trash[V], e16[V], AFT.Square,
                            accum_out=part[V, COL_E + blk * C + c: COL_E + blk * C + c + 1])

                    # ---- constrain per z ----
                    for z in range(3):
                        cA = tpool.tile([TP, CHK], F16, tag="cA")
                        nc.vector.scalar_tensor_tensor(
                            cA[V], gneg[V, z, :], -2.0 / 3.0, fdy16[V, 3 + z, :],
                            AOT.mult, AOT.add)
                        dzo = tpool.tile([TP, CHK], F16, tag="dzo")
                        nc.vector.tensor_sub(
                            dzo[V], o16[V, _zc(z, 1), :], o16[V, _zc(z, -1), :])
                        cB = tpool.tile([TP, CHK], F16, tag="cB")
                        nc.vector.scalar_tensor_tensor(
                            cB[V], dzo[V], 0.5, cA[V], AOT.mult, AOT.add)
                        trash2 = tpool.tile([TP, CHK], F16, tag="trash2")
                        nc.vector.scalar_tensor_tensor(
                            trash2[V], cB[V], 1.0, cB[V], AOT.mult, AOT.mult,
                            accum_out=part[V, COL_CON + blk * 3 + z: COL_CON + blk * 3 + z + 1])

            nc.sync.dma_start(PART[:], part[:])

    nc.compile()
    return nc


def kernel(Field, W, b_bias, thermal_factor):
    Field = np.asarray(Field, np.float32)
    W = np.asarray(W, np.float32)
    b_bias = np.asarray(b_bias, np.float32)
    thermal_factor = np.asarray(thermal_factor, np.float32)

    if "nc" not in _CACHE:
        _CACHE["nc"] = build_nc()
    nc = _CACHE["nc"]

    consts = _host_consts(W, b_bias, thermal_factor)
    in_maps = []
    for k in range(NCORES):
        bk, h = k // 2, k % 2
        rows = np.arange(h * YS - 2, h * YS + YS + 2) % Y
        shard = Field[bk][:, rows, :]
        m = {"F": np.ascontiguousarray(shard)}
        m.update(consts)
        in_maps.append(m)

    res = bass_utils.run_bass_kernel_spmd(nc, in_maps, core_ids=list(range(NCORES)))

    out = np.empty((B, C, Y, X), np.float32)
    sums = np.zeros(4, np.float64)
    for k in range(NCORES):
        bk, h = k // 2, k % 2
        out[bk, :, h * YS:(h + 1) * YS, :] = res.results[k]["FN"]
        part = res.results[k]["PART"].astype(np.float64)
        sums[0] += part[:, COL_CON:COL_CON + NBLK * NCH * 3].sum()
        sums[1] += part[:, COL_E:COL_E + NPER].sum()
        sums[2] += part[:, COL_PP:COL_PP + NPER].sum()
        sums[3] += part[:, COL_XY:COL_XY + NPER].sum()

    n12 = float(B * C * Y * X)
    n3 = float(B * 3 * Y * X)
    return (
        out,
        np.float32(sums[0] / n3),
        np.float32(sums[1] / n12),
        np.float32(sums[2] / n12),
        np.float32(sums[3] / n12),
    )


# revision 22
# speedup vs baseline: 1.3572x; 1.3572x over previous
"""Trainium2 Bass kernel for nn_EulerEquationModel (Euler equation step).

Contract: kernel(**inputs) takes FULL inputs (Field (4,12,720,1440) f32,
W (15,12), b_bias (15,), thermal_factor (1,)) and returns the full output
tuple matching reference():
  (Field_new (4,12,720,1440) f32, mean(constrain^2), mean(E^2),
   mean(PhysicsPart^2), mean(xydir^2))

Sharding: 8 cores = 4 batches x 2 y-halves (360 rows each + 2-row circular
y-halo and 2-col circular x-pad supplied by the host). Params replicated.

Per-core plan (shard F: (12, 364, 1444) f32, x pre-padded circularly):
  - y-blocks of 120 output rows; block tiles keep the 4 halo rows at
    partitions 120..123 so every compute op starts at partition 0
    (hardware constraint); stencil lhsT rows are permuted to match.
  - Loads are fp32->fp16 casting SWDGE DMAs straight into fp16 tiles.
  - PE (fp16): Fdy banded-stencil matmul; Fdz via +/-0.5*I over z-neighbor
    channels; channel mix twice per 8-row sub-block in an interleaved
    (yl, c) layout: once with identity folded (F+E, o) and once bare (E)
    whose PSUM is square-accumulated by ScalarE for mean(E^2).
  - Mix outputs return to y-partition layout via a DRAM fp16 round-trip
    (SBUF->SBUF partition remaps defeat the Tile dependency tracker).
  - DVE/GpSimd (fp16): x-derivative taps (d1 on GpSimd since odd shifts
    break DVE 2x alignment), advection products/sums, final combine.
  - mean-square reductions: ScalarE Square+accum_out; finished on host.
"""

import numpy as np

import concourse.bacc as bacc
import concourse.mybir as mybir
import concourse.tile as tile
from concourse import bass_utils

AOT = mybir.AluOpType
AFT = mybir.ActivationFunctionType
F32 = mybir.dt.float32
F16 = mybir.dt.float16

B, C, Y, X = 4, 12, 720, 1440
NCORES = 8
YS = Y // 2          # 360 rows per core
YSH = YS + 4         # 364 shard rows (2-row halo each side)
XP = X + 4           # 1444 padded cols (2-col circular pad each side)
NBLK = 3             # y-blocks per core
BLK = 120            # output rows per block
TP = 124             # tile partitions (BLK + 4 halo rows at 120..123)
NCH = 2              # x-chunks
CHK = X // NCH       # 720
CW = CHK + 4         # 724 tile cols
NO = 15              # mix output channels (12 Enew + 3 o)
MIXK = 96            # mix contraction: 8 yl * 12 c
MIXM = 120           # mix out: 8 yl * 15 o
MIX2M = 96           # bare-E mix out: 8 yl * 12 o
NSB = BLK // 8       # 15 sub-blocks per block

NBC = NBLK * NCH               # 6 block-chunks
COL_XY = 0                     # 72: (blk*12+c) sum(xydir^2)
COL_PP = NBC * C               # 72: sum(PhysicsPart^2)
COL_CON = 2 * NBC * C          # 18: (blk*3+z) sum(constrain^2)
COL_E = 2 * NBC * C + NBC * 3  # 90: (blk*15+sb) sum(E^2)
NCOLS = COL_E + NBC * NSB

_CACHE = {}


def _host_consts(W, b_bias, thermal_factor):
    w5a, w5b = 8.0 / 12.0, 1.0 / 12.0
    mix = np.zeros((MIXK, MIXM), np.float32)
    mix2 = np.zeros((MIXK, MIX2M), np.float32)
    for yl in range(8):
        for o in range(NO):
            for c in range(C):
                mix[yl * 12 + c, yl * 15 + o] = W[o, c] + (1.0 if o == c else 0.0)
                if o < C:
                    mix2[yl * 12 + c, yl * 12 + o] = W[o, c]
    # Fdy banded stencil lhsT [TP, BLK]; rhs partition k holds block row
    # r(k): k<120 -> 2+k, k=120..123 -> halo rows 0,1,122,123.
    rowmap = list(range(2, 2 + BLK)) + [0, 1, BLK + 2, BLK + 3]
    sfy = np.zeros((TP, BLK), np.float32)
    taps = {-2: w5b, -1: -w5a, 1: w5a, 2: -w5b}
    for k in range(TP):
        for m in range(BLK):
            d = rowmap[k] - (2 + m)
            if d in taps:
                sfy[k, m] = taps[d]
    halfI = np.zeros((TP, BLK), np.float32)
    for m in range(BLK):
        halfI[m, m] = 0.5
    b_aug = np.zeros((MIXM, 1), np.float32)
    b2_aug = np.zeros((MIX2M, 1), np.float32)
    for yl in range(8):
        for o in range(NO):
            b_aug[yl * 15 + o, 0] = b_bias[o]
            if o < C:
                b2_aug[yl * 12 + o, 0] = b_bias[o]
    p_list = np.array([10.0, 8.5, 5.0], np.float32)
    tfp = np.tile((thermal_factor[0] / p_list)[None, :], (TP, 1)).astype(np.float32)
    return {
        "lhsT_mix": mix.astype(np.float16),
        "lhsT_mix2": mix2.astype(np.float16),
        "lhsT_sfy": sfy.astype(np.float16),
        "lhsT_hI": halfI.astype(np.float16),
        "lhsT_nhI": (-halfI).astype(np.float16),
        "b_aug": b_aug,
        "b2_aug": b2_aug,
        "tfp": tfp,
    }


def _zc(z, d):
    return (z + d) % 3


def build_nc():
    nc = bacc.Bacc("TRN2", target_bir_lowering=False, debug=False,
                   num_devices=NCORES)

    F = nc.dram_tensor("F", [C, YSH, XP], F32, kind="ExternalInput").ap()
    lhsT_mix = nc.dram_tensor("lhsT_mix", [MIXK, MIXM], F16, kind="ExternalInput").ap()
    lhsT_mix2 = nc.dram_tensor("lhsT_mix2", [MIXK, MIX2M], F16, kind="ExternalInput").ap()
    lhsT_sfy = nc.dram_tensor("lhsT_sfy", [TP, BLK], F16, kind="ExternalInput").ap()
    lhsT_hI = nc.dram_tensor("lhsT_hI", [TP, BLK], F16, kind="ExternalInput").ap()
    lhsT_nhI = nc.dram_tensor("lhsT_nhI", [TP, BLK], F16, kind="ExternalInput").ap()
    b_aug_d = nc.dram_tensor("b_aug", [MIXM, 1], F32, kind="ExternalInput").ap()
    b2_aug_d = nc.dram_tensor("b2_aug", [MIX2M, 1], F32, kind="ExternalInput").ap()
    tfp_d = nc.dram_tensor("tfp", [TP, 3], F32, kind="ExternalInput").ap()

    FN = nc.dram_tensor("FN", [C, YS, X], F32, kind="ExternalOutput").ap()
    PART = nc.dram_tensor("PART", [TP, NCOLS], F32, kind="ExternalOutput").ap()

    with tile.TileContext(nc) as tc:
        with (
            tc.tile_pool(name="consts", bufs=1) as cpool,
            tc.tile_pool(name="x16p", bufs=1) as x16pool,
            tc.tile_pool(name="l2p", bufs=1) as l2pool,
            tc.tile_pool(name="ml2", bufs=1) as ml2pool,
            tc.tile_pool(name="big16", bufs=1) as bigpool,
            tc.tile_pool(name="tmp", bufs=2) as tpool,
            tc.tile_pool(name="fnp", bufs=3) as fnpool,
            tc.tile_pool(name="partp", bufs=1) as partpool,
            tc.tile_pool(name="dramp", bufs=2, space="DRAM") as dramp,
            tc.tile_pool(name="ps_fy", bufs=1, space="PSUM") as psfy,
            tc.tile_pool(name="ps_fz", bufs=1, space="PSUM") as psfz,
            tc.tile_pool(name="ps_mx", bufs=1, space="PSUM") as psmx,
        ):
            c_mix = cpool.tile([MIXK, MIXM], F16, tag="c_mix")
            nc.sync.dma_start(c_mix[:], lhsT_mix[:])
            c_mix2 = cpool.tile([MIXK, MIX2M], F16, tag="c_mix2")
            nc.sync.dma_start(c_mix2[:], lhsT_mix2[:])
            c_sfy = cpool.tile([TP, BLK], F16, tag="c_sfy")
            nc.sync.dma_start(c_sfy[:], lhsT_sfy[:])
            c_hI = cpool.tile([TP, BLK], F16, tag="c_hI")
            nc.sync.dma_start(c_hI[:], lhsT_hI[:])
            c_nhI = cpool.tile([TP, BLK], F16, tag="c_nhI")
            nc.sync.dma_start(c_nhI[:], lhsT_nhI[:])
            c_bias = cpool.tile([MIXM, 1], F32, tag="c_bias")
            nc.sync.dma_start(c_bias[:], b_aug_d[:])
            c_bias2 = cpool.tile([MIX2M, 1], F32, tag="c_bias2")
            nc.sync.dma_start(c_bias2[:], b2_aug_d[:])
            c_tfp = cpool.tile([TP, 3], F32, tag="c_tfp")
            nc.sync.dma_start(c_tfp[:], tfp_d[:])

            part = partpool.tile([TP, NCOLS], F32, tag="part")
            nc.vector.memset(part[:], 0.0)

            for j in range(NBLK):
                y0 = j * BLK
                for i in range(NCH):
                    x0 = i * CHK  # padded-col of tile col 0
                    blk = j * NCH + i
                    H = CHK // 2  # matmul half-width (PSUM bank limit)

                    # ---- fp16 loads (casting SWDGE DMAs) ----
                    x16 = x16pool.tile([TP, C, CW], F16, tag="x16")
                    nc.gpsimd.dma_start(
                        x16[0:BLK],
                        F[:, y0 + 2: y0 + 2 + BLK, x0: x0 + CW]
                        .rearrange("c y x -> y c x"))
                    nc.gpsimd.dma_start(
                        x16[BLK:BLK + 2],
                        F[:, y0: y0 + 2, x0: x0 + CW]
                        .rearrange("c y x -> y c x"))
                    nc.gpsimd.dma_start(
                        x16[BLK + 2:BLK + 4],
                        F[:, y0 + BLK + 2: y0 + BLK + 4, x0: x0 + CW]
                        .rearrange("c y x -> y c x"))

                    l2 = l2pool.tile([MIXK, NSB, CHK], F16, tag="l2")
                    for yl in range(8):
                        nc.gpsimd.dma_start(
                            l2[yl * C: (yl + 1) * C],
                            F[:, y0 + 2 + yl: y0 + 2 + BLK: 8,
                              x0 + 2: x0 + 2 + CHK])

                    # ---- PE: channel mix per 8-row sub-block ----
                    enew16 = bigpool.tile([TP, C, CHK], F16, tag="enew16",
                                          padded_shape=[TP, C + 1, CHK])
                    o16 = bigpool.tile([TP, 3, CHK], F16, tag="o16",
                                       padded_shape=[TP, 4, CHK])
                    strip = ml2pool.tile([MIXM, NSB, CHK], F16, tag="strip")
                    for sb in range(NSB):
                        rhs = l2[:, sb, :]
                        mps = psmx.tile([MIXM, 2, 512], F32, tag="mps")
                        for h in range(2):
                            nc.tensor.matmul(mps[:, h, 0:H], c_mix[:],
                                             rhs[:, h * H: (h + 1) * H],
                                             start=True, stop=True)
                        nc.scalar.activation(
                            strip[:, sb, :], mps[:, :, 0:H],
                            AFT.Identity, bias=c_bias[:], scale=1.0)
                        # bare-E mix: square-accumulated straight from PSUM
                        mps2 = psmx.tile([MIX2M, 2, 512], F32, tag="mps2")
                        for h in range(2):
                            nc.tensor.matmul(mps2[:, h, 0:H], c_mix2[:],
                                             rhs[:, h * H: (h + 1) * H],
                                             start=True, stop=True)
                        tr96 = tpool.tile([MIX2M, CHK], F16, tag="tr96")
                        nc.scalar.activation(
                            tr96[:], mps2[:, :, 0:H],
                            AFT.Square, bias=c_bias2[:], scale=1.0,
                            accum_out=part[0:MIX2M,
                                           COL_E + blk * NSB + sb:
                                           COL_E + blk * NSB + sb + 1])

                    # remap via DRAM round-trip (plain APs only)
                    e2 = dramp.tile([NO, BLK, CHK], F16, tag="e2")
                    e2r = e2[:].rearrange("o (sb yl) x -> yl o sb x", yl=8)
                    for yl in range(8):
                        nc.sync.dma_start(
                            e2r[yl], strip[yl * NO: (yl + 1) * NO, :, :])
                    nc.sync.dma_start(
                        enew16[0:BLK], e2[0:C].rearrange("c y x -> y c x"))
                    nc.sync.dma_start(
                        o16[0:BLK], e2[C:NO].rearrange("z y x -> y z x"))

                    # ---- PE: Fdy (one lhsT) ----
                    fdy16 = bigpool.tile([TP, C, CHK], F16, tag="fdy16")
                    for c in range(C):
                        fyp = psfy.tile([BLK, 2, 512], F32, tag="fyp")
                        for h in range(2):
                            nc.tensor.matmul(
                                fyp[:, h, 0:H], c_sfy[:],
                                x16[:, c, 2 + h * H: 2 + (h + 1) * H],
                                start=True, stop=True)
                        nc.scalar.activation(
                            fdy16[0:BLK, c, :],
                            fyp[:, :, 0:H], AFT.Copy)

                    # ---- PE: Fdz = 0.5*(x[zp] - x[zm]) ----
                    fdz16 = bigpool.tile([TP, C, CHK], F16, tag="fdz16")
                    for c in range(C):
                        s, z = c // 3, c % 3
                        zp = 3 * s + _zc(z, 1)
                        zm = 3 * s + _zc(z, -1)
                        fzp = psfz.tile([BLK, 2, 512], F32, tag="fzp")
                        for h in range(2):
                            w = slice(2 + h * H, 2 + (h + 1) * H)
                            nc.tensor.matmul(fzp[:, h, 0:H], c_hI[:], x16[:, zp, w],
                                             start=True, stop=False)
                            nc.tensor.matmul(fzp[:, h, 0:H], c_nhI[:], x16[:, zm, w],
                                             start=False, stop=True)
                        nc.scalar.activation(
                            fdz16[0:BLK, c, :],
                            fzp[:, :, 0:H], AFT.Copy)

                    # ---- stencil prep: d1 (gpsimd), d2+gneg (DVE) ----
                    gneg = bigpool.tile([TP, C, CHK], F16, tag="gneg")
                    for c in range(C):
                        d1t = tpool.tile([TP, CHK], F16, tag="d1")
                        nc.gpsimd.tensor_sub(
                            d1t[:], x16[:, c, 3: 3 + CHK], x16[:, c, 1: 1 + CHK])
                        d2t = tpool.tile([TP, CHK], F16, tag="d2")
                        nc.vector.tensor_sub(
                            d2t[:], x16[:, c, 4: 4 + CHK], x16[:, c, 0: CHK])
                        # gneg = d2/8 - d1  (Fdx = -(2/3)*gneg)
                        nc.vector.scalar_tensor_tensor(
                            gneg[:, c, :], d2t[:], 0.125, d1t[:],
                            AOT.mult, AOT.subtract)

                    # ---- prep: uneg = -(2/3)u, term3 = (tf/p)*T*o ----
                    uneg = tpool.tile([TP, 3, CHK], F16, tag="uneg")
                    nc.vector.tensor_scalar_mul(
                        uneg[:], x16[:, 0:3, 2: 2 + CHK], -2.0 / 3.0)
                    term3 = tpool.tile([TP, 3, CHK], F16, tag="term3")
                    for z in range(3):
                        tprime = tpool.tile([TP, CHK], F16, tag="tprime")
                        nc.vector.tensor_scalar(
                            tprime[0:BLK], x16[0:BLK, 6 + z, 2: 2 + CHK],
                            c_tfp[0:BLK, z:z + 1], None, AOT.mult)
                        nc.vector.tensor_mul(
                            term3[0:BLK, z, :], tprime[0:BLK], o16[0:BLK, z, :])

                    # ---- nonlinear chain per channel ----
                    V = slice(0, BLK)
                    for c in range(C):
                        s, z = c // 3, c % 3
                        A = tpool.tile([TP, CHK], F16, tag="A")
                        nc.vector.tensor_mul(A[V], gneg[V, c, :], uneg[V, z, :])
                        Bt = tpool.tile([TP, CHK], F16, tag="Bt")
                        nc.vector.tensor_mul(
                            Bt[V], fdy16[V, c, :], x16[V, 3 + z, 2: 2 + CHK])
                        S1 = tpool.tile([TP, CHK], F16, tag="S1")
                        nc.vector.tensor_add(S1[V], A[V], Bt[V])
                        Ct = tpool.tile([TP, CHK], F16, tag="Ct")
                        nc.vector.tensor_mul(Ct[V], fdz16[V, c, :], o16[V, z, :])
                        q2 = tpool.tile([TP, CHK], F16, tag="q2")
                        nc.vector.tensor_add(q2[V], S1[V], Ct[V])
                        trash = tpool.tile([TP, CHK], F16, tag="trash")
                        # r1: sum(xydir^2) = sum(S1^2)
                        nc.scalar.activation(
                            trash[V], S1[V], AFT.Square,
                            accum_out=part[V, COL_XY + blk * C + c:
                                           COL_XY + blk * C + c + 1])
                        # q3 = q2 - phys_c
                        if s == 0:
                            q3 = tpool.tile([TP, CHK], F16, tag="q3")
                            nc.vector.scalar_tensor_tensor(
                                q3[V], gneg[V, 9 + z, :], -2.0 / 3.0, q2[V],
                                AOT.mult, AOT.add)
                        elif s == 1:
                            q3 = tpool.tile([TP, CHK], F16, tag="q3")
                            nc.vector.tensor_add(q3[V], q2[V], fdy16[V, 9 + z, :])
                        elif s == 2:
                            q3 = tpool.tile([TP, CHK], F16, tag="q3")
                            nc.vector.tensor_sub(q3[V], q2[V], term3[V, z, :])
                        else:
                            q3 = q2
                        # r2: sum(PhysicsPart^2) = sum(q3^2)
                        nc.scalar.activation(
                            trash[V], q3[V], AFT.Square,
                            accum_out=part[V, COL_PP + blk * C + c:
                                           COL_PP + blk * C + c + 1])
                        # FN = Enew - q3 (fp32 out)
                        fn = fnpool.tile([TP, CHK], F32, tag="fn")
                        nc.vector.tensor_sub(fn[V], enew16[V, c, :], q3[V])
                        nc.sync.dma_start(
                            FN[c, y0: y0 + BLK, x0: x0 + CHK], fn[V])

                    # ---- constrain per z ----
                    for z in range(3):
                        cA = tpool.tile([TP, CHK], F16, tag="cA")
                        nc.vector.scalar_tensor_tensor(
                            cA[V], gneg[V, z, :], -2.0 / 3.0, fdy16[V, 3 + z, :],
                            AOT.mult, AOT.add)
                        dzo = tpool.tile([TP, CHK], F16, tag="dzo")
                        nc.vector.tensor_sub(
                            dzo[V], o16[V, _zc(z, 1), :], o16[V, _zc(z, -1), :])
                        cB = tpool.tile([TP, CHK], F16, tag="cB")
                        nc.vector.scalar_tensor_tensor(
                            cB[V], dzo[V], 0.5, cA[V], AOT.mult, AOT.add)
                        trash2 = tpool.tile([TP, CHK], F16, tag="trash2")
                        nc.vector.scalar_tensor_tensor(
                            trash2[V], cB[V], 1.0, cB[V], AOT.mult, AOT.mult,
                            accum_out=part[V, COL_CON + blk * 3 + z:
                                           COL_CON + blk * 3 + z + 1])

            nc.sync.dma_start(PART[:], part[:])

    nc.compile()
    return nc


def kernel(Field, W, b_bias, thermal_factor):
    Field = np.asarray(Field, np.float32)
    W = np.asarray(W, np.float32)
    b_bias = np.asarray(b_bias, np.float32)
    thermal_factor = np.asarray(thermal_factor, np.float32)

    if "nc" not in _CACHE:
        _CACHE["nc"] = build_nc()
    nc = _CACHE["nc"]

    consts = _host_consts(W, b_bias, thermal_factor)
    in_maps = []
    rows_all = [np.arange(h * YS - 2, h * YS + YS + 2) % Y for h in range(2)]
    cols = np.arange(-2, X + 2) % X
    for k in range(NCORES):
        bk, h = k // 2, k % 2
        shard = Field[bk][:, rows_all[h][:, None], cols[None, :]]
        m = {"F": np.ascontiguousarray(shard)}
        m.update(consts)
        in_maps.append(m)

    res = bass_utils.run_bass_kernel_spmd(nc, in_maps, core_ids=list(range(NCORES)))

    out = np.empty((B, C, Y, X), np.float32)
    sums = np.zeros(4, np.float64)
    for k in range(NCORES):
        bk, h = k // 2, k % 2
        out[bk, :, h * YS:(h + 1) * YS, :] = res.results[k]["FN"]
        part = res.results[k]["PART"].astype(np.float64)
        sums[0] += part[:, COL_CON:COL_CON + NBC * 3].sum()
        sums[1] += part[:, COL_E:COL_E + NBC * NSB].sum()
        sums[2] += part[:, COL_PP:COL_PP + NBC * C].sum()
        sums[3] += part[:, COL_XY:COL_XY + NBC * C].sum()

    n12 = float(B * C * Y * X)
    n3 = float(B * 3 * Y * X)
    return (
        out,
        np.float32(sums[0] / n3),
        np.float32(sums[1] / n12),
        np.float32(sums[2] / n12),
        np.float32(sums[3] / n12),
    )


# revision 27
# speedup vs baseline: 1.3614x; 1.0030x over previous
"""Trainium2 Bass kernel for nn_EulerEquationModel (Euler equation step).

Contract: kernel(**inputs) takes FULL inputs (Field (4,12,720,1440) f32,
W (15,12), b_bias (15,), thermal_factor (1,)) and returns the full output
tuple matching reference():
  (Field_new (4,12,720,1440) f32, mean(constrain^2), mean(E^2),
   mean(PhysicsPart^2), mean(xydir^2))

Sharding: 8 cores = 4 batches x 2 y-halves (360 rows each + 2-row circular
y-halo and 2-col circular x-pad supplied by the host). Params replicated.

Per-core plan (shard F: (12, 364, 1444) f32, x pre-padded circularly):
  - y-blocks of 120 output rows; block tiles keep the 4 halo rows at
    partitions 120..123 so every compute op starts at partition 0
    (hardware constraint); stencil lhsT rows are permuted to match.
  - Loads are fp32->fp16 casting SWDGE DMAs straight into fp16 tiles.
  - PE (fp16): Fdy banded-stencil matmul; Fdz via +/-0.5*I over z-neighbor
    channels; channel mix twice per 8-row sub-block in an interleaved
    (yl, c) layout: once with identity folded (F+E, o) and once bare (E)
    whose PSUM is square-accumulated by ScalarE for mean(E^2).
  - Mix outputs return to y-partition layout via a DRAM fp16 round-trip
    (SBUF->SBUF partition remaps defeat the Tile dependency tracker).
  - DVE/GpSimd (fp16): x-derivative taps (d1 on GpSimd since odd shifts
    break DVE 2x alignment), advection products/sums, final combine.
  - mean-square reductions: ScalarE Square+accum_out; finished on host.
"""

import numpy as np

import concourse.bacc as bacc
import concourse.mybir as mybir
import concourse.tile as tile
from concourse import bass_utils

AOT = mybir.AluOpType
AFT = mybir.ActivationFunctionType
F32 = mybir.dt.float32
F16 = mybir.dt.float16

B, C, Y, X = 4, 12, 720, 1440
NCORES = 8
YS = Y // 2          # 360 rows per core
YSH = YS + 4         # 364 shard rows (2-row halo each side)
XP = X + 4           # 1444 padded cols (2-col circular pad each side)
NBLK = 3             # y-blocks per core
BLK = 120            # output rows per block
TP = 124             # tile partitions (BLK + 4 halo rows at 120..123)
NCH = 2              # x-chunks
CHK = X // NCH       # 720
CW = CHK + 4         # 724 tile cols
NO = 15              # mix output channels (12 Enew + 3 o)
MIXK = 96            # mix contraction: 8 yl * 12 c
MIXM = 120           # mix out: 8 yl * 15 o
MIX2M = 96           # bare-E mix out: 8 yl * 12 o
NSB = BLK // 8       # 15 sub-blocks per block

NBC = NBLK * NCH               # 6 block-chunks
COL_XY = 0                     # 72: (blk*12+c) sum(xydir^2)
COL_PP = NBC * C               # 72: sum(PhysicsPart^2)
COL_CON = 2 * NBC * C          # 18: (blk*3+z) sum(constrain^2)
COL_E = 2 * NBC * C + NBC * 3  # 90: (blk*15+sb) sum(E^2)
NCOLS = COL_E + NBC * NSB

_CACHE = {}


def _host_consts(W, b_bias, thermal_factor):
    w5a, w5b = 8.0 / 12.0, 1.0 / 12.0
    mix = np.zeros((MIXK, MIXM), np.float32)
    mix2 = np.zeros((MIXK, MIX2M), np.float32)
    for yl in range(8):
        for o in range(NO):
            for c in range(C):
                mix[yl * 12 + c, yl * 15 + o] = W[o, c] + (1.0 if o == c else 0.0)
                if o < C:
                    mix2[yl * 12 + c, yl * 12 + o] = W[o, c]
    # Fdy banded stencil lhsT [TP, BLK]; rhs partition k holds block row
    # r(k): k<120 -> 2+k, k=120..123 -> halo rows 0,1,122,123.
    rowmap = list(range(2, 2 + BLK)) + [0, 1, BLK + 2, BLK + 3]
    sfy = np.zeros((TP, BLK), np.float32)
    taps = {-2: w5b, -1: -w5a, 1: w5a, 2: -w5b}
    for k in range(TP):
        for m in range(BLK):
            d = rowmap[k] - (2 + m)
            if d in taps:
                sfy[k, m] = taps[d]
    halfI = np.zeros((TP, BLK), np.float32)
    for m in range(BLK):
        halfI[m, m] = 0.5
    b_aug = np.zeros((MIXM, 1), np.float32)
    b2_aug = np.zeros((MIX2M, 1), np.float32)
    for yl in range(8):
        for o in range(NO):
            b_aug[yl * 15 + o, 0] = b_bias[o]
            if o < C:
                b2_aug[yl * 12 + o, 0] = b_bias[o]
    p_list = np.array([10.0, 8.5, 5.0], np.float32)
    tfp = np.tile((thermal_factor[0] / p_list)[None, :], (TP, 1)).astype(np.float32)
    return {
        "lhsT_mix": mix.astype(np.float16),
        "lhsT_mix2": mix2.astype(np.float16),
        "lhsT_sfy": sfy.astype(np.float16),
        "lhsT_hI": halfI.astype(np.float16),
        "lhsT_nhI": (-halfI).astype(np.float16),
        "b_aug": b_aug,
        "b2_aug": b2_aug,
        "tfp": tfp,
    }


def _zc(z, d):
    return (z + d) % 3


def build_nc():
    nc = bacc.Bacc("TRN2", target_bir_lowering=False, debug=False,
                   num_devices=NCORES)

    F = nc.dram_tensor("F", [C, YSH, XP], F32, kind="ExternalInput").ap()
    lhsT_mix = nc.dram_tensor("lhsT_mix", [MIXK, MIXM], F16, kind="ExternalInput").ap()
    lhsT_mix2 = nc.dram_tensor("lhsT_mix2", [MIXK, MIX2M], F16, kind="ExternalInput").ap()
    lhsT_sfy = nc.dram_tensor("lhsT_sfy", [TP, BLK], F16, kind="ExternalInput").ap()
    lhsT_hI = nc.dram_tensor("lhsT_hI", [TP, BLK], F16, kind="ExternalInput").ap()
    lhsT_nhI = nc.dram_tensor("lhsT_nhI", [TP, BLK], F16, kind="ExternalInput").ap()
    b_aug_d = nc.dram_tensor("b_aug", [MIXM, 1], F32, kind="ExternalInput").ap()
    b2_aug_d = nc.dram_tensor("b2_aug", [MIX2M, 1], F32, kind="ExternalInput").ap()
    tfp_d = nc.dram_tensor("tfp", [TP, 3], F32, kind="ExternalInput").ap()

    FN = nc.dram_tensor("FN", [C, YS, X], F32, kind="ExternalOutput").ap()
    PART = nc.dram_tensor("PART", [TP, NCOLS], F32, kind="ExternalOutput").ap()

    with tile.TileContext(nc) as tc:
        with (
            tc.tile_pool(name="consts", bufs=1) as cpool,
            tc.tile_pool(name="x16p", bufs=2) as x16pool,
            tc.tile_pool(name="l2p", bufs=1) as l2pool,
            tc.tile_pool(name="ml2", bufs=1) as ml2pool,
            tc.tile_pool(name="big16", bufs=1) as bigpool,
            tc.tile_pool(name="tmp", bufs=2) as tpool,
            tc.tile_pool(name="fnp", bufs=2) as fnpool,
            tc.tile_pool(name="partp", bufs=1) as partpool,
            tc.tile_pool(name="prep", bufs=1) as ppool,
            tc.tile_pool(name="dramp", bufs=2, space="DRAM") as dramp,
            tc.tile_pool(name="ps_st", bufs=2, space="PSUM") as psst,
            tc.tile_pool(name="ps_mx", bufs=2, space="PSUM") as psmx,
        ):
            c_mix = cpool.tile([MIXK, MIXM], F16, tag="c_mix")
            nc.sync.dma_start(c_mix[:], lhsT_mix[:])
            c_mix2 = cpool.tile([MIXK, MIX2M], F16, tag="c_mix2")
            nc.sync.dma_start(c_mix2[:], lhsT_mix2[:])
            c_sfy = cpool.tile([TP, BLK], F16, tag="c_sfy")
            nc.sync.dma_start(c_sfy[:], lhsT_sfy[:])
            c_hI = cpool.tile([TP, BLK], F16, tag="c_hI")
            nc.sync.dma_start(c_hI[:], lhsT_hI[:])
            c_nhI = cpool.tile([TP, BLK], F16, tag="c_nhI")
            nc.sync.dma_start(c_nhI[:], lhsT_nhI[:])
            c_bias = cpool.tile([MIXM, 1], F32, tag="c_bias")
            nc.sync.dma_start(c_bias[:], b_aug_d[:])
            c_bias2 = cpool.tile([MIX2M, 1], F32, tag="c_bias2")
            nc.sync.dma_start(c_bias2[:], b2_aug_d[:])
            c_tfp = cpool.tile([TP, 3], F32, tag="c_tfp")
            nc.sync.dma_start(c_tfp[:], tfp_d[:])

            part = partpool.tile([TP, NCOLS], F32, tag="part")
            nc.vector.memset(part[:], 0.0)

            for j in range(NBLK):
                y0 = j * BLK
                for i in range(NCH):
                    x0 = i * CHK  # padded-col of tile col 0
                    blk = j * NCH + i
                    H = CHK // 2  # matmul half-width (PSUM bank limit)

                    # ---- fp16 loads (casting SWDGE DMAs) ----
                    x16 = x16pool.tile([TP, C, CW], F16, tag="x16")
                    nc.gpsimd.dma_start(
                        x16[0:BLK],
                        F[:, y0 + 2: y0 + 2 + BLK, x0: x0 + CW]
                        .rearrange("c y x -> y c x"))
                    nc.gpsimd.dma_start(
                        x16[BLK:BLK + 2],
                        F[:, y0: y0 + 2, x0: x0 + CW]
                        .rearrange("c y x -> y c x"))
                    nc.gpsimd.dma_start(
                        x16[BLK + 2:BLK + 4],
                        F[:, y0 + BLK + 2: y0 + BLK + 4, x0: x0 + CW]
                        .rearrange("c y x -> y c x"))

                    l2 = l2pool.tile([MIXK, NSB, CHK], F16, tag="l2")
                    for yl in range(8):
                        nc.gpsimd.dma_start(
                            l2[yl * C: (yl + 1) * C],
                            F[:, y0 + 2 + yl: y0 + 2 + BLK: 8,
                              x0 + 2: x0 + 2 + CHK])

                    # ---- PE: channel mix per 8-row sub-block ----
                    enew16 = bigpool.tile([TP, C, CHK], F16, tag="enew16",
                                          padded_shape=[TP, C + 1, CHK])
                    o16 = bigpool.tile([TP, 3, CHK], F16, tag="o16",
                                       padded_shape=[TP, 4, CHK])
                    strip = ml2pool.tile([MIXM, NSB, CHK], F16, tag="strip")
                    for sb in range(NSB):
                        rhs = l2[:, sb, :]
                        mps = psmx.tile([MIXM, 2, 512], F32, tag="mps")
                        for h in range(2):
                            nc.tensor.matmul(mps[:, h, 0:H], c_mix[:],
                                             rhs[:, h * H: (h + 1) * H],
                                             start=True, stop=True)
                        nc.scalar.activation(
                            strip[:, sb, :], mps[:, :, 0:H],
                            AFT.Identity, bias=c_bias[:], scale=1.0)
                        # bare-E mix: square-accumulated straight from PSUM
                        mps2f = psmx.tile([MIXM, 2, 512], F32, tag="mps")
                        mps2 = mps2f[0:MIX2M]
                        for h in range(2):
                            nc.tensor.matmul(mps2[:, h, 0:H], c_mix2[:],
                                             rhs[:, h * H: (h + 1) * H],
                                             start=True, stop=True)
                        tr96 = tpool.tile([MIX2M, CHK], F16, tag="tr96")
                        nc.scalar.activation(
                            tr96[:], mps2[:, :, 0:H],
                            AFT.Square, bias=c_bias2[:], scale=1.0,
                            accum_out=part[0:MIX2M,
                                           COL_E + blk * NSB + sb:
                                           COL_E + blk * NSB + sb + 1])

                    # remap via DRAM round-trip (plain APs only)
                    e2 = dramp.tile([NO, BLK, CHK], F16, tag="e2")
                    e2r = e2[:].rearrange("o (sb yl) x -> yl o sb x", yl=8)
                    for yl in range(8):
                        nc.sync.dma_start(
                            e2r[yl], strip[yl * NO: (yl + 1) * NO, :, :])
                    nc.sync.dma_start(
                        enew16[0:BLK], e2[0:C].rearrange("c y x -> y c x"))
                    nc.sync.dma_start(
                        o16[0:BLK], e2[C:NO].rearrange("z y x -> y z x"))

                    # ---- PE: Fdy (one lhsT) ----
                    fdy16 = bigpool.tile([TP, C, CHK], F16, tag="fdy16")
                    for c in range(C):
                        fyp = psst.tile([BLK, 2, 512], F32, tag="stp")
                        for h in range(2):
                            nc.tensor.matmul(
                                fyp[:, h, 0:H], c_sfy[:],
                                x16[:, c, 2 + h * H: 2 + (h + 1) * H],
                                start=True, stop=True)
                        nc.scalar.activation(
                            fdy16[0:BLK, c, :],
                            fyp[:, :, 0:H], AFT.Copy)

                    # ---- PE: Fdz = 0.5*(x[zp] - x[zm]) ----
                    fdz16 = bigpool.tile([TP, C, CHK], F16, tag="fdz16")
                    for c in range(C):
                        s, z = c // 3, c % 3
                        zp = 3 * s + _zc(z, 1)
                        zm = 3 * s + _zc(z, -1)
                        fzp = psst.tile([BLK, 2, 512], F32, tag="stp")
                        for h in range(2):
                            w = slice(2 + h * H, 2 + (h + 1) * H)
                            nc.tensor.matmul(fzp[:, h, 0:H], c_hI[:], x16[:, zp, w],
                                             start=True, stop=False)
                            nc.tensor.matmul(fzp[:, h, 0:H], c_nhI[:], x16[:, zm, w],
                                             start=False, stop=True)
                        nc.scalar.activation(
                            fdz16[0:BLK, c, :],
                            fzp[:, :, 0:H], AFT.Copy)

                    # ---- stencil prep: d1 (gpsimd), d2+gneg (DVE) ----
                    gneg = bigpool.tile([TP, C, CHK], F16, tag="gneg")
                    for c in range(C):
                        d1t = tpool.tile([TP, CHK], F16, tag="d1")
                        nc.gpsimd.tensor_sub(
                            d1t[:], x16[:, c, 3: 3 + CHK], x16[:, c, 1: 1 + CHK])
                        d2t = tpool.tile([TP, CHK], F16, tag="d2")
                        nc.gpsimd.tensor_sub(
                            d2t[:], x16[:, c, 4: 4 + CHK], x16[:, c, 0: CHK])
                        # gneg = d2/8 - d1  (Fdx = -(2/3)*gneg)
                        nc.vector.scalar_tensor_tensor(
                            gneg[:, c, :], d2t[:], 0.125, d1t[:],
                            AOT.mult, AOT.subtract)

                    # ---- prep: uneg = -(2/3)u, term3 = (tf/p)*T*o ----
                    uneg = ppool.tile([TP, 3, CHK], F16, tag="uneg")
                    nc.vector.tensor_scalar_mul(
                        uneg[:], x16[:, 0:3, 2: 2 + CHK], -2.0 / 3.0)
                    term3 = ppool.tile([TP, 3, CHK], F16, tag="term3")
                    for z in range(3):
                        tprime = ppool.tile([TP, CHK], F16, tag="tprime")
                        nc.vector.tensor_scalar(
                            tprime[0:BLK], x16[0:BLK, 6 + z, 2: 2 + CHK],
                            c_tfp[0:BLK, z:z + 1], None, AOT.mult)
                        nc.vector.tensor_mul(
                            term3[0:BLK, z, :], tprime[0:BLK], o16[0:BLK, z, :])

                    # ---- nonlinear chain per channel ----
                    V = slice(0, BLK)
                    for c in range(C):
                        s, z = c // 3, c % 3
                        A = tpool.tile([TP, CHK], F16, tag="A")
                        nc.vector.tensor_mul(A[V], gneg[V, c, :], uneg[V, z, :])
                        Bt = tpool.tile([TP, CHK], F16, tag="Bt")
                        nc.vector.tensor_mul(
                            Bt[V], fdy16[V, c, :], x16[V, 3 + z, 2: 2 + CHK])
                        S1 = tpool.tile([TP, CHK], F16, tag="S1")
                        nc.vector.tensor_add(S1[V], A[V], Bt[V])
                        Ct = tpool.tile([TP, CHK], F16, tag="Ct")
                        nc.vector.tensor_mul(Ct[V], fdz16[V, c, :], o16[V, z, :])
                        q2 = tpool.tile([TP, CHK], F16, tag="q2")
                        nc.vector.tensor_add(q2[V], S1[V], Ct[V])
                        trash = tpool.tile([TP, CHK], F16, tag="trash")
                        # r1: sum(xydir^2) = sum(S1^2)
                        nc.scalar.activation(
                            trash[V], S1[V], AFT.Square,
                            accum_out=part[V, COL_XY + blk * C + c:
                                           COL_XY + blk * C + c + 1])
                        # q3 = q2 - phys_c
                        if s == 0:
                            q3 = tpool.tile([TP, CHK], F16, tag="q3")
                            nc.vector.scalar_tensor_tensor(
                                q3[V], gneg[V, 9 + z, :], -2.0 / 3.0, q2[V],
                                AOT.mult, AOT.add)
                        elif s == 1:
                            q3 = tpool.tile([TP, CHK], F16, tag="q3")
                            nc.vector.tensor_add(q3[V], q2[V], fdy16[V, 9 + z, :])
                        elif s == 2:
                            q3 = tpool.tile([TP, CHK], F16, tag="q3")
                            nc.vector.tensor_sub(q3[V], q2[V], term3[V, z, :])
                        else:
                            q3 = q2
                        # r2: sum(PhysicsPart^2) = sum(q3^2)
                        nc.scalar.activation(
                            trash[V], q3[V], AFT.Square,
                            accum_out=part[V, COL_PP + blk * C + c:
                                           COL_PP + blk * C + c + 1])
                        # FN = Enew - q3 (fp32 out)
                        fn = fnpool.tile([TP, CHK], F32, tag="fn")
                        nc.vector.tensor_sub(fn[V], enew16[V, c, :], q3[V])
                        nc.sync.dma_start(
                            FN[c, y0: y0 + BLK, x0: x0 + CHK], fn[V])

                    # ---- constrain per z ----
                    for z in range(3):
                        cA = tpool.tile([TP, CHK], F16, tag="cA")
                        nc.vector.scalar_tensor_tensor(
                            cA[V], gneg[V, z, :], -2.0 / 3.0, fdy16[V, 3 + z, :],
                            AOT.mult, AOT.add)
                        dzo = tpool.tile([TP, CHK], F16, tag="dzo")
                        nc.vector.tensor_sub(
                            dzo[V], o16[V, _zc(z, 1), :], o16[V, _zc(z, -1), :])
                        cB = tpool.tile([TP, CHK], F16, tag="cB")
                        nc.vector.scalar_tensor_tensor(
                            cB[V], dzo[V], 0.5, cA[V], AOT.mult, AOT.add)
                        trash2 = tpool.tile([TP, CHK], F16, tag="trash2")
                        nc.vector.scalar_tensor_tensor(
                            trash2[V], cB[V], 1.0, cB[V], AOT.mult, AOT.mult,
                            accum_out=part[V, COL_CON + blk * 3 + z:
                                           COL_CON + blk * 3 + z + 1])

            nc.sync.dma_start(PART[:], part[:])

    nc.compile()
    return nc


def kernel(Field, W, b_bias, thermal_factor):
    Field = np.asarray(Field, np.float32)
    W = np.asarray(W, np.float32)
    b_bias = np.asarray(b_bias, np.float32)
    thermal_factor = np.asarray(thermal_factor, np.float32)

    if "nc" not in _CACHE:
        _CACHE["nc"] = build_nc()
    nc = _CACHE["nc"]

    consts = _host_consts(W, b_bias, thermal_factor)
    in_maps = []
    rows_all = [np.arange(h * YS - 2, h * YS + YS + 2) % Y for h in range(2)]
    cols = np.arange(-2, X + 2) % X
    for k in range(NCORES):
        bk, h = k // 2, k % 2
        shard = Field[bk][:, rows_all[h][:, None], cols[None, :]]
        m = {"F": np.ascontiguousarray(shard)}
        m.update(consts)
        in_maps.append(m)

    res = bass_utils.run_bass_kernel_spmd(nc, in_maps, core_ids=list(range(NCORES)))

    out = np.empty((B, C, Y, X), np.float32)
    sums = np.zeros(4, np.float64)
    for k in range(NCORES):
        bk, h = k // 2, k % 2
        out[bk, :, h * YS:(h + 1) * YS, :] = res.results[k]["FN"]
        part = res.results[k]["PART"].astype(np.float64)
        sums[0] += part[:, COL_CON:COL_CON + NBC * 3].sum()
        sums[1] += part[:, COL_E:COL_E + NBC * NSB].sum()
        sums[2] += part[:, COL_PP:COL_PP + NBC * C].sum()
        sums[3] += part[:, COL_XY:COL_XY + NBC * C].sum()

    n12 = float(B * C * Y * X)
    n3 = float(B * 3 * Y * X)
    return (
        out,
        np.float32(sums[0] / n3),
        np.float32(sums[1] / n12),
        np.float32(sums[2] / n12),
        np.float32(sums[3] / n12),
    )
